# revision 4
# baseline (speedup 1.0000x reference)
"""DeepSpeed-style MLP block (pre-LN residual add + LN + GEMM+GELU + GEMM +
residual) for Trainium2, data-parallel over tokens across 8 NeuronCores.

fp8 (e4m3) DoubleRow variant: both GEMMs run with perf_mode=DoubleRow (2 fp8
weights per PE cell, K=256 per matmul) at ~2x the bf16 per-K rate. To keep the
fp8 quantization error well inside the 2e-2 gate, the GELU is split into a
linear part and a small nonlinear residual:

    h@W2 = g*@W2 + x@(W1'W2)/2,   g* = gelu(z) - (z - b1)/2

The fp8 stream carries only g* (~2.3x smaller than h, so ~2.3x less
quantization noise) while the linear half rides a bf16 GEMM against the
host-precomputed W12 = W1'@W2 (K=1024, 1/4 the FLOPs of GEMM2). The
W1-quantization noise similarly only enters through (gelu' - 1/2), not gelu'.
Host-side sim: rel err 1.05e-2 (vs 1.79e-2 for plain fp8, 1.1e-3 for bf16);
measured on HW: 1.047e-2.

Per-core schedule (tokens sharded 8 x 4096, processed in 512-token blocks).
GEMM1 m-chunks are interleaved with GEMM2-n0 k-pairs so the PE stream covers
the ACT(gelu) + DVE(g*) latency per m; block tb+1's LN chunks are emitted
every 4th m-pair to keep the DVE fed just-in-time; transposes are batched 8
per PSUM bank and evicted with ONE cast per (g); the pre-LN adds and the
residual carry run on GPSIMD (SBUF-only ops) to unload the DVE.

PE per block: 128 DR (GEMM1) + 128 DR + 64 bf16 (GEMM2+W12) + 32 transposes
~= 76us; measured DR/bf16 issue rate 216 ns at N=512.
"""

import sys

sys.path.insert(0, "/opt/trn_rl_repo")

import numpy as np
import ml_dtypes

import concourse.bass as bass
import concourse.mybir as mybir
import concourse.tile as tile
from concourse.masks import make_identity
from concourse.bass_utils import run_bass_kernel_spmd

AFT = mybir.ActivationFunctionType
ALU = mybir.AluOpType
DR = mybir.MatmulPerfMode.DoubleRow
FP32 = mybir.dt.float32
BF16 = mybir.dt.bfloat16
FP8 = mybir.dt.float8e4

N_CORES = 8
B, S, H, I = 4, 8192, 1024, 4096
NTOK = B * S              # 32768 tokens total
T = NTOK // N_CORES       # 4096 tokens per core
TB = 512                  # tokens per block (moving free dim)
G = TB // 128             # 4 token sub-tiles per block
KH = H // 128             # 8 contraction chunks for GEMM1 / W12 GEMM
MI = I // 128             # 32 I-chunks (GEMM1 out / GEMM2 contraction)
NH = H // 512             # 2 H output slices for GEMM2
EPS = 1e-5

S_X = 16.0                # fp8 scale on the LN output x
S_W = 1024.0              # fp8 scale on W1 and W2
C1 = 1.0 / (S_X * S_W)    # GEMM1 psum -> z
USE_W12 = True            # gelu split + bf16 W12 correction GEMM


def _split_multiwait_instructions(nc):
    """This walrus build accepts only ONE sync-wait command per instruction.
    Move extra waits onto fresh same-engine NOPs placed just before the
    offending instruction."""
    n_split = 0
    for f in nc.m.functions:
        for bb in f.blocks:
            insts = list(bb.instructions)
            new = []
            changed = False
            for inst in insts:
                si = inst.sync_info
                if si is not None and si.on_wait and len(si.on_wait) > 1:
                    waits = list(si.on_wait)
                    for w in waits[:-1]:
                        nop = mybir.InstNoOp(name=nc.get_next_instruction_name())
                        nop.engine = inst.engine
                        nop.sync_info = mybir.SyncInfo(on_wait=[w], on_update=[])
                        new.append(nop)
                        n_split += 1
                    si.on_wait = waits[-1:]
                    changed = True
                new.append(inst)
            if changed:
                bb.instructions = new
    return n_split


def _bcast_ap(ap, p=128):
    """AP view of a DRAM vector broadcast across p partitions."""
    return bass.AP(tensor=ap.tensor, offset=ap.offset, ap=[[0, p]] + list(ap.ap))


def _build(n_blocks=T // TB, use_w12=USE_W12):
    nc = bass.Bass("TRN2")
    t_rows = n_blocks * TB
    xin = nc.declare_dram_parameter("xin", [t_rows, H], FP32, isOutput=False)
    res = nc.declare_dram_parameter("res", [t_rows, H], FP32, isOutput=False)
    w1 = nc.declare_dram_parameter("w1", [H, I], FP8, isOutput=False)
    w2 = nc.declare_dram_parameter("w2", [I, H], FP8, isOutput=False)
    if use_w12:
        w12 = nc.declare_dram_parameter("w12", [H, H], BF16, isOutput=False)
    b1c = nc.declare_dram_parameter("b1c", [128, MI], FP32, isOutput=False)
    b2v = nc.declare_dram_parameter("b2v", [H], FP32, isOutput=False)
    out = nc.declare_dram_parameter("out", [t_rows, H], FP32, isOutput=True)

    with tile.TileContext(nc) as tc:
        with (
            tc.tile_pool(name="const", bufs=1) as const,
            tc.tile_pool(name="ing", bufs=2) as ing,
            tc.tile_pool(name="tmpg", bufs=4) as tmpg,
            tc.tile_pool(name="blk1", bufs=1) as blk1,
            tc.tile_pool(name="blk2", bufs=2) as blk2,
            tc.tile_pool(name="htmp", bufs=3) as htmp,
            tc.tile_pool(name="outp", bufs=4) as outp,
            tc.tile_pool(name="statp", bufs=2) as statp,
            tc.tile_pool(name="ps1", bufs=2, space="PSUM") as ps1,
            tc.tile_pool(name="ps2", bufs=4, space="PSUM") as ps2,
            tc.tile_pool(name="pst", bufs=2, space="PSUM") as pst,
        ):
            b2_bc = const.tile([128, H], FP32)
            nc.gpsimd.dma_start(out=b2_bc, in_=_bcast_ap(b2v[:]))
            b1_sb = const.tile([128, MI], FP32)
            nc.gpsimd.dma_start(out=b1_sb, in_=b1c[:, :])
            eps_t = const.tile([128, 1], FP32)
            nc.vector.memset(eps_t, EPS / (S_X * S_X))
            ident = const.tile([128, 128], BF16)
            make_identity(nc, ident)

            # ---- LN pipeline, split into per-g chunks + finish + transposes
            # so the main loop can interleave them into the GEMM stream ----

            def ln_alloc(tb):
                st = {
                    "x0": blk1.tile([128, G, H], BF16, name=f"x0_{tb}", tag="x0"),
                    "xT8": blk1.tile([128, KH, TB], FP8, name=f"xT8_{tb}", tag="xT8"),
                    "xTb": (
                        blk1.tile([128, KH, TB], BF16, name=f"xTb_{tb}", tag="xTb")
                        if use_w12
                        else None
                    ),
                    "r32": blk2.tile([128, G, H], FP32, name=f"r32_{tb}", tag="r32"),
                    "mvb": statp.tile([128, G, 2], FP32, name=f"mvb_{tb}", tag="mvb"),
                    "rstd": statp.tile([128, G], FP32, name=f"rsd_{tb}", tag="rstd"),
                    "tmp": [None] * G,
                    "stats": [None] * G,
                }
                return st

            def ln_chunk(tb, g, st):
                """DMA + pre-LN add + bn stats for one 128-token group."""
                t0 = tb * TB
                ra, rb = t0 + g * 128, t0 + (g + 1) * 128
                xin_g = ing.tile([128, H], FP32, name=f"xin_{tb}_{g}", tag="xin")
                res_g = ing.tile([128, H], FP32, name=f"res_{tb}_{g}", tag="res")
                nc.sync.dma_start(out=xin_g, in_=xin[ra:rb, :])
                nc.sync.dma_start(out=res_g, in_=res[ra:rb, :])
                tmp = tmpg.tile([128, H], FP32, name=f"tmp_{tb}_{g}", tag="tmp")
                nc.gpsimd.tensor_add(out=tmp, in0=xin_g, in1=res_g)
                # residual carry r + output_b (consumed by the evicts)
                nc.gpsimd.tensor_add(out=st["r32"][:, g, :], in0=tmp, in1=b2_bc)
                stats = statp.tile([128, 2, 6], FP32, name=f"st_{tb}_{g}", tag="stats")
                tmp_r = tmp.rearrange("p (s d) -> p s d", s=2)
                for s_ in range(2):
                    nc.vector.bn_stats(out=stats[:, s_, :], in_=tmp_r[:, s_, :])
                nc.vector.bn_aggr(out=st["mvb"][:, g, :], in_=stats)
                st["tmp"][g] = tmp

            def ln_finish(tb, st):
                """Batched rstd (one ACT table load) + x0 writes."""
                # sqrt((var+eps)/S_X^2) then reciprocal -> S_X * rsqrt(var+eps)
                nc.scalar.activation(
                    out=st["rstd"], in_=st["mvb"][:, :, 1], func=AFT.Sqrt,
                    bias=eps_t, scale=1.0 / (S_X * S_X),
                )
                nc.vector.reciprocal(out=st["rstd"], in_=st["rstd"])
                for g in range(G):
                    nc.vector.tensor_scalar(
                        out=st["x0"][:, g, :],
                        in0=st["tmp"][g],
                        scalar1=st["mvb"][:, g, 0:1],
                        scalar2=st["rstd"][:, g : g + 1],
                        op0=ALU.subtract,
                        op1=ALU.mult,
                    )

            def ln_transposes(tb, st):
                """PE transposes (batched 8 per PSUM bank) + one cast per g."""
                for g in range(G):
                    ptg = pst.tile([128, KH, 128], BF16, name=f"pt_{tb}_{g}", tag="pt")
                    for k in range(KH):
                        nc.tensor.transpose(
                            ptg[:, k, :], st["x0"][:, g, k * 128 : (k + 1) * 128], ident
                        )
                    nc.vector.tensor_copy(
                        out=st["xT8"][:, :, g * 128 : (g + 1) * 128], in_=ptg
                    )
                    if use_w12:
                        nc.vector.tensor_copy(
                            out=st["xTb"][:, :, g * 128 : (g + 1) * 128], in_=ptg
                        )

            def emit_g1_m(tb, m, tiles):
                """GEMM1 for one m-chunk: 4 DR matmuls + gelu + g* (or plain h)."""
                p1 = ps1.tile([128, TB], FP32, name=f"p1_{tb}_{m}", tag="p1")
                for k in range(KH // 2):
                    nc.tensor.matmul(
                        p1,
                        lhsT=w1_sb[:, 2 * k : 2 * k + 2, m * 128 : (m + 1) * 128],
                        rhs=tiles["xT8"][:, 2 * k : 2 * k + 2, :],
                        start=(k == 0),
                        stop=(k == KH // 2 - 1),
                        perf_mode=DR,
                    )
                hT = tiles["hT"]
                if use_w12:
                    h_t = htmp.tile([128, TB], BF16, name=f"ht_{tb}_{m}", tag="ht")
                    nc.scalar.activation(
                        out=h_t, in_=p1, func=AFT.Gelu_apprx_tanh,
                        bias=b1_sb[:, m : m + 1], scale=C1,
                    )
                    nc.vector.scalar_tensor_tensor(
                        out=hT[:, m, :], in0=p1, scalar=-C1 / 2, in1=h_t,
                        op0=ALU.mult, op1=ALU.add,
                    )
                else:
                    nc.scalar.activation(
                        out=hT[:, m, :], in_=p1, func=AFT.Gelu_apprx_tanh,
                        bias=b1_sb[:, m : m + 1], scale=C1,
                    )

            def emit_g2_dr_k(tb, n, k, p2s, tiles, start, stop):
                hT = tiles["hT"]
                for g in range(G):
                    nc.tensor.matmul(
                        p2s[g],
                        lhsT=hT[:, 2 * k : 2 * k + 2, g * 128 : (g + 1) * 128],
                        rhs=w2_sb[:, 2 * k : 2 * k + 2, n * 512 : (n + 1) * 512],
                        start=start,
                        stop=stop,
                        perf_mode=DR,
                    )

            def emit_w12(tb, n, p2s, tiles, start, stop):
                xTb = tiles["xTb"]
                for k in range(KH):
                    for g in range(G):
                        nc.tensor.matmul(
                            p2s[g],
                            lhsT=xTb[:, k, g * 128 : (g + 1) * 128],
                            rhs=w12_sb[:, k, n * 512 : (n + 1) * 512],
                            start=start and (k == 0),
                            stop=stop and (k == KH - 1),
                        )

            def emit_evict(tb, n, p2s, tiles):
                t0 = tb * TB
                for g in range(G):
                    o = outp.tile([128, 512], FP32, name=f"o_{tb}_{n}_{g}", tag="o")
                    nc.vector.scalar_tensor_tensor(
                        out=o,
                        in0=p2s[g],
                        scalar=1.0 / S_W,
                        in1=tiles["r32"][:, g, n * 512 : (n + 1) * 512],
                        op0=ALU.mult,
                        op1=ALU.add,
                    )
                    nc.gpsimd.dma_start(
                        out=out[t0 + g * 128 : t0 + (g + 1) * 128, n * 512 : (n + 1) * 512],
                        in_=o,
                    )

            # ---- preamble: LN for block 0, then weight preloads ----
            st0 = ln_alloc(0)
            for g in range(G):
                ln_chunk(0, g, st0)
            w1_sb = const.tile([128, KH, I], FP8, name="w1_sb")
            w2_sb = const.tile([128, MI, H], FP8, name="w2_sb")
            w12_sb = (
                const.tile([128, KH, H], BF16, name="w12_sb") if use_w12 else None
            )
            for k in range(KH):
                nc.sync.dma_start(out=w1_sb[:, k, :], in_=w1[k * 128 : (k + 1) * 128, :])
            for ks in range(4):
                nc.sync.dma_start(
                    out=w2_sb[:, ks * 8 : (ks + 1) * 8, :],
                    in_=w2[ks * 8 * 128 : (ks + 1) * 8 * 128, :].rearrange(
                        "(k p) h -> p k h", p=128
                    ),
                )
            if use_w12:
                nc.sync.dma_start(
                    out=w12_sb,
                    in_=w12[:, :].rearrange("(k p) h -> p k h", p=128),
                )
            ln_finish(0, st0)
            ln_transposes(0, st0)
            tiles = {"xT8": st0["xT8"], "xTb": st0["xTb"], "r32": st0["r32"]}

            # ---- main block loop ----
            for tb in range(n_blocks):
                tiles["hT"] = blk1.tile([128, MI, TB], FP8, name=f"hT_{tb}", tag="hT")
                st_next = ln_alloc(tb + 1) if tb + 1 < n_blocks else None
                # n=0 accumulation group: GEMM1 m-pairs interleaved with the
                # GEMM2 DR k-pairs (start), W12 term appended (stop).
                p2s0 = [
                    ps2.tile([128, 512], FP32, name=f"p2_{tb}_0_{g}", tag="p2")
                    for g in range(G)
                ]
                for mp in range(MI // 2):
                    emit_g1_m(tb, 2 * mp, tiles)
                    emit_g1_m(tb, 2 * mp + 1, tiles)
                    emit_g2_dr_k(
                        tb, 0, mp, p2s0, tiles,
                        start=(mp == 0),
                        stop=(not use_w12) and (mp == MI // 2 - 1),
                    )
                    if st_next is not None and mp % 4 == 3:
                        ln_chunk(tb + 1, mp // 4, st_next)
                if st_next is not None:
                    ln_finish(tb + 1, st_next)
                if use_w12:
                    emit_w12(tb, 0, p2s0, tiles, start=False, stop=True)
                emit_evict(tb, 0, p2s0, tiles)
                # n=1 group: W12 first (start), then transposes for block tb+1,
                # then the DR k-pairs (stop).
                p2s1 = [
                    ps2.tile([128, 512], FP32, name=f"p2_{tb}_1_{g}", tag="p2")
                    for g in range(G)
                ]
                if use_w12:
                    emit_w12(tb, 1, p2s1, tiles, start=True, stop=False)
                if st_next is not None:
                    ln_transposes(tb + 1, st_next)
                for k in range(MI // 2):
                    emit_g2_dr_k(
                        tb, 1, k, p2s1, tiles,
                        start=(not use_w12) and (k == 0),
                        stop=(k == MI // 2 - 1),
                    )
                emit_evict(tb, 1, p2s1, tiles)
                if st_next is not None:
                    tiles = {
                        "xT8": st_next["xT8"],
                        "xTb": st_next["xTb"],
                        "r32": st_next["r32"],
                    }

    return nc


def _prep_inputs(input, residual, bias, attn_nw, attn_nb, inter_w, inter_b, output_w, output_b, use_w12=USE_W12):
    """Host-side preprocessing: fold bias into the input stream and the LN
    affine into W1/b1, scale + cast weights to fp8 e4m3 (clip to +-240: TRN
    e4m3 overflows to inf), precompute W12 = W1'@W2 in bf16, shard tokens."""
    f8 = ml_dtypes.float8_e4m3
    bf = ml_dtypes.bfloat16
    biasf = np.asarray(bias, np.float32)
    x2 = np.ascontiguousarray(
        np.asarray(input, np.float32).reshape(NTOK, H) + biasf
    )
    r2 = np.ascontiguousarray(np.asarray(residual, np.float32).reshape(NTOK, H))
    gamma = np.asarray(attn_nw, np.float64)
    beta = np.asarray(attn_nb, np.float64)
    w1f = np.asarray(inter_w, np.float64)
    w2f = np.asarray(output_w, np.float64)
    w1p = gamma[:, None] * w1f
    w1b = np.ascontiguousarray(
        np.clip(w1p * S_W, -240, 240).astype(np.float32).astype(f8)
    )
    b1p = (np.asarray(inter_b, np.float64) + beta @ w1f).astype(np.float32)
    b1c = np.ascontiguousarray(b1p.reshape(MI, 128).T)
    w2b = np.ascontiguousarray(
        np.clip(w2f * S_W, -240, 240).astype(np.float32).astype(f8)
    )
    b2f = np.asarray(output_b, np.float32)

    in_maps = []
    for c in range(N_CORES):
        sl = slice(c * T, (c + 1) * T)
        im = {
            "xin": x2[sl],
            "res": r2[sl],
            "w1": w1b,
            "w2": w2b,
            "b1c": b1c,
            "b2v": b2f,
        }
        in_maps.append(im)
    if use_w12:
        w12b = np.ascontiguousarray(
            ((w1p @ w2f) * (S_W / (2 * S_X))).astype(np.float32).astype(bf)
        )
        for im in in_maps:
            im["w12"] = w12b
    return in_maps


def _run(inputs, trace=False, **kwargs):
    in_maps = _prep_inputs(
        inputs["input"],
        inputs["residual"],
        inputs["bias"],
        inputs["attn_nw"],
        inputs["attn_nb"],
        inputs["inter_w"],
        inputs["inter_b"],
        inputs["output_w"],
        inputs["output_b"],
    )
    nc = _build()
    _split_multiwait_instructions(nc)
    r = run_bass_kernel_spmd(nc, in_maps, list(range(N_CORES)), trace=trace, **kwargs)
    outs = [r.results[c]["out"] for c in range(N_CORES)]
    full = np.concatenate(outs, axis=0).reshape(B, S, H).astype(np.float32)
    return full, r


def kernel(**inputs):
    out, _ = _run(inputs, trace=False)
    return out


if __name__ == "__main__":
    nc = _build(1)
    print("built 1-block variant ok:", len(nc.m.functions[0].blocks))


# revision 9
# speedup vs baseline: 1.1045x; 1.1045x over previous
"""DeepSpeed-style MLP block (pre-LN residual add + LN + GEMM+GELU + GEMM +
residual) for Trainium2, data-parallel over tokens across 8 NeuronCores.

fp8 (e4m3) DoubleRow variant: both GEMMs run with perf_mode=DoubleRow (2 fp8
weights per PE cell, K=256 per matmul) at ~2x the bf16 per-K rate. To keep the
fp8 quantization error well inside the 2e-2 gate, the GELU is split into a
linear part and a small nonlinear residual:

    h@W2 = g*@W2 + x@(W1'W2)/2,   g* = gelu(z) - (z - b1)/2

The fp8 stream carries only g* (~2.3x smaller than h, so ~2.3x less
quantization noise) while the linear half rides a bf16 GEMM against the
host-precomputed W12 = W1'@W2 (K=1024, 1/4 the FLOPs of GEMM2). The
W1-quantization noise similarly only enters through (gelu' - 1/2), not gelu'.
Host-side sim: rel err 1.05e-2 (vs 1.79e-2 for plain fp8, 1.1e-3 for bf16);
measured on HW: 1.047e-2.

Per-core schedule (tokens sharded 8 x 4096, processed in 512-token blocks).
GEMM1 m-chunks are interleaved with GEMM2-n0 k-pairs so the PE stream covers
the ACT(gelu) + DVE(g*) latency per m; block tb+1's LN chunks are emitted
every 4th m-pair to keep the DVE fed just-in-time; transposes are batched 8
per PSUM bank and evicted with ONE cast per (g); the pre-LN adds and the
residual carry run on GPSIMD (SBUF-only ops) to unload the DVE.

PE per block: 128 DR (GEMM1) + 128 DR + 64 bf16 (GEMM2+W12) + 32 transposes
~= 76us; measured DR/bf16 issue rate 216 ns at N=512.
"""

import sys

sys.path.insert(0, "/opt/trn_rl_repo")

import numpy as np
import ml_dtypes

import concourse.bass as bass
import concourse.mybir as mybir
import concourse.tile as tile
from concourse.masks import make_identity
from concourse.bass_utils import run_bass_kernel_spmd

AFT = mybir.ActivationFunctionType
ALU = mybir.AluOpType
DR = mybir.MatmulPerfMode.DoubleRow
FP32 = mybir.dt.float32
BF16 = mybir.dt.bfloat16
FP8 = mybir.dt.float8e4

N_CORES = 8
B, S, H, I = 4, 8192, 1024, 4096
NTOK = B * S              # 32768 tokens total
T = NTOK // N_CORES       # 4096 tokens per core
TB = 512                  # tokens per block (moving free dim)
G = TB // 128             # 4 token sub-tiles per block
KH = H // 128             # 8 contraction chunks for GEMM1 / W12 GEMM
MI = I // 128             # 32 I-chunks (GEMM1 out / GEMM2 contraction)
NH = H // 512             # 2 H output slices for GEMM2
EPS = 1e-5

S_X = 16.0                # fp8 scale on the LN output x
S_W = 1024.0              # fp8 scale on W1 and W2
C1 = 1.0 / (S_X * S_W)    # GEMM1 psum -> z
USE_W12 = True            # gelu split + bf16 W12 correction GEMM


def _split_multiwait_instructions(nc):
    """This walrus build accepts only ONE sync-wait command per instruction.
    Move extra waits onto fresh same-engine NOPs placed just before the
    offending instruction."""
    n_split = 0
    for f in nc.m.functions:
        for bb in f.blocks:
            insts = list(bb.instructions)
            new = []
            changed = False
            for inst in insts:
                si = inst.sync_info
                if si is not None and si.on_wait and len(si.on_wait) > 1:
                    waits = list(si.on_wait)
                    for w in waits[:-1]:
                        nop = mybir.InstNoOp(name=nc.get_next_instruction_name())
                        nop.engine = inst.engine
                        nop.sync_info = mybir.SyncInfo(on_wait=[w], on_update=[])
                        new.append(nop)
                        n_split += 1
                    si.on_wait = waits[-1:]
                    changed = True
                new.append(inst)
            if changed:
                bb.instructions = new
    return n_split


def _bcast_ap(ap, p=128):
    """AP view of a DRAM vector broadcast across p partitions."""
    return bass.AP(tensor=ap.tensor, offset=ap.offset, ap=[[0, p]] + list(ap.ap))


def _build(n_blocks=T // TB, use_w12=USE_W12):
    nc = bass.Bass("TRN2")
    t_rows = n_blocks * TB
    xin = nc.declare_dram_parameter("xin", [t_rows, H], FP32, isOutput=False)
    res = nc.declare_dram_parameter("res", [t_rows, H], FP32, isOutput=False)
    w1 = nc.declare_dram_parameter("w1", [H, I], FP8, isOutput=False)
    w2 = nc.declare_dram_parameter("w2", [I, H], FP8, isOutput=False)
    if use_w12:
        w12 = nc.declare_dram_parameter("w12", [H, H], BF16, isOutput=False)
    b1c = nc.declare_dram_parameter("b1c", [128, MI], FP32, isOutput=False)
    b2v = nc.declare_dram_parameter("b2v", [H], FP32, isOutput=False)
    out = nc.declare_dram_parameter("out", [t_rows, H], FP32, isOutput=True)

    with tile.TileContext(nc) as tc:
        with (
            tc.tile_pool(name="const", bufs=1) as const,
            tc.tile_pool(name="ing", bufs=2) as ing,
            tc.tile_pool(name="tmpg", bufs=4) as tmpg,
            tc.tile_pool(name="blk1", bufs=1) as blk1,
            tc.tile_pool(name="blk2", bufs=2) as blk2,
            tc.tile_pool(name="htmp", bufs=3) as htmp,
            tc.tile_pool(name="outp", bufs=4) as outp,
            tc.tile_pool(name="statp", bufs=2) as statp,
            tc.tile_pool(name="ps1", bufs=2, space="PSUM") as ps1,
            tc.tile_pool(name="ps2", bufs=4, space="PSUM") as ps2,
            tc.tile_pool(name="pst", bufs=2, space="PSUM") as pst,
        ):
            b2_bc = const.tile([128, H], FP32)
            nc.gpsimd.dma_start(out=b2_bc, in_=_bcast_ap(b2v[:]))
            b1_sb = const.tile([128, MI], FP32)
            nc.gpsimd.dma_start(out=b1_sb, in_=b1c[:, :])
            eps_t = const.tile([128, 1], FP32)
            nc.vector.memset(eps_t, EPS / (S_X * S_X))
            ident = const.tile([128, 128], BF16)
            make_identity(nc, ident)

            # ---- LN pipeline, split into per-g chunks + finish + transposes
            # so the main loop can interleave them into the GEMM stream ----

            def ln_alloc(tb):
                st = {
                    "x0": blk1.tile([128, G, H], BF16, name=f"x0_{tb}", tag="x0"),
                    "xT8": blk1.tile([128, KH, TB], FP8, name=f"xT8_{tb}", tag="xT8"),
                    "xTb": (
                        blk1.tile([128, KH, TB], BF16, name=f"xTb_{tb}", tag="xTb")
                        if use_w12
                        else None
                    ),
                    "r32": blk2.tile([128, G, H], FP32, name=f"r32_{tb}", tag="r32"),
                    "mvb": statp.tile([128, G, 2], FP32, name=f"mvb_{tb}", tag="mvb"),
                    "rstd": statp.tile([128, G], FP32, name=f"rsd_{tb}", tag="rstd"),
                    "tmp": [None] * G,
                    "stats": [None] * G,
                }
                return st

            def ln_chunk(tb, g, st):
                """DMA + pre-LN add + bn stats for one 128-token group."""
                t0 = tb * TB
                ra, rb = t0 + g * 128, t0 + (g + 1) * 128
                xin_g = ing.tile([128, H], FP32, name=f"xin_{tb}_{g}", tag="xin")
                res_g = ing.tile([128, H], FP32, name=f"res_{tb}_{g}", tag="res")
                nc.sync.dma_start(out=xin_g, in_=xin[ra:rb, :])
                nc.sync.dma_start(out=res_g, in_=res[ra:rb, :])
                tmp = tmpg.tile([128, H], FP32, name=f"tmp_{tb}_{g}", tag="tmp")
                nc.vector.tensor_add(out=tmp, in0=xin_g, in1=res_g)
                # residual carry r + output_b (consumed by the evicts; gpsimd
                # is slow (~2.4us) but this is off the critical LN chain)
                nc.gpsimd.tensor_add(out=st["r32"][:, g, :], in0=tmp, in1=b2_bc)
                stats = statp.tile([128, 2, 6], FP32, name=f"st_{tb}_{g}", tag="stats")
                tmp_r = tmp.rearrange("p (s d) -> p s d", s=2)
                for s_ in range(2):
                    nc.vector.bn_stats(out=stats[:, s_, :], in_=tmp_r[:, s_, :])
                nc.vector.bn_aggr(out=st["mvb"][:, g, :], in_=stats)
                st["tmp"][g] = tmp

            def ln_finish(tb, st, gs=None):
                """rstd (one ACT table load for the batch) + x0 writes."""
                gs = range(G) if gs is None else gs
                gsl = slice(gs[0], gs[-1] + 1)
                # sqrt((var+eps)/S_X^2) then reciprocal -> S_X * rsqrt(var+eps)
                nc.scalar.activation(
                    out=st["rstd"][:, gsl], in_=st["mvb"][:, gsl, 1], func=AFT.Sqrt,
                    bias=eps_t, scale=1.0 / (S_X * S_X),
                )
                nc.vector.reciprocal(out=st["rstd"][:, gsl], in_=st["rstd"][:, gsl])
                for g in gs:
                    nc.vector.tensor_scalar(
                        out=st["x0"][:, g, :],
                        in0=st["tmp"][g],
                        scalar1=st["mvb"][:, g, 0:1],
                        scalar2=st["rstd"][:, g : g + 1],
                        op0=ALU.subtract,
                        op1=ALU.mult,
                    )

            def ln_transposes(tb, st, gs=None):
                """PE transposes (batched 8 per PSUM bank) + one cast per g."""
                for g in range(G) if gs is None else gs:
                    ptg = pst.tile([128, KH, 128], BF16, name=f"pt_{tb}_{g}", tag="pt")
                    for k in range(KH):
                        nc.tensor.transpose(
                            ptg[:, k, :], st["x0"][:, g, k * 128 : (k + 1) * 128], ident
                        )
                    nc.vector.tensor_copy(
                        out=st["xT8"][:, :, g * 128 : (g + 1) * 128], in_=ptg
                    )
                    if use_w12:
                        nc.vector.tensor_copy(
                            out=st["xTb"][:, :, g * 128 : (g + 1) * 128], in_=ptg
                        )

            def emit_g1_m(tb, m, tiles):
                """GEMM1 for one m-chunk: 4 DR matmuls + gelu + g* (or plain h)."""
                p1 = ps1.tile([128, TB], FP32, name=f"p1_{tb}_{m}", tag="p1")
                for k in range(KH // 2):
                    nc.tensor.matmul(
                        p1,
                        lhsT=w1_sb[:, 2 * k : 2 * k + 2, m * 128 : (m + 1) * 128],
                        rhs=tiles["xT8"][:, 2 * k : 2 * k + 2, :],
                        start=(k == 0),
                        stop=(k == KH // 2 - 1),
                        perf_mode=DR,
                    )
                hT = tiles["hT"]
                if use_w12:
                    h_t = htmp.tile([128, TB], BF16, name=f"ht_{tb}_{m}", tag="ht")
                    nc.scalar.activation(
                        out=h_t, in_=p1, func=AFT.Gelu_apprx_tanh,
                        bias=b1_sb[:, m : m + 1], scale=C1,
                    )
                    nc.vector.scalar_tensor_tensor(
                        out=hT[:, m, :], in0=p1, scalar=-C1 / 2, in1=h_t,
                        op0=ALU.mult, op1=ALU.add,
                    )
                else:
                    nc.scalar.activation(
                        out=hT[:, m, :], in_=p1, func=AFT.Gelu_apprx_tanh,
                        bias=b1_sb[:, m : m + 1], scale=C1,
                    )

            def emit_g2_dr_k(tb, n, k, p2s, tiles, start, stop):
                hT = tiles["hT"]
                for g in range(G):
                    nc.tensor.matmul(
                        p2s[g],
                        lhsT=hT[:, 2 * k : 2 * k + 2, g * 128 : (g + 1) * 128],
                        rhs=w2_sb[:, 2 * k : 2 * k + 2, n * 512 : (n + 1) * 512],
                        start=start,
                        stop=stop,
                        perf_mode=DR,
                    )

            def emit_w12_g(tb, n, g, p2s, tiles, start, stop):
                xTb = tiles["xTb"]
                for k in range(KH):
                    nc.tensor.matmul(
                        p2s[g],
                        lhsT=xTb[:, k, g * 128 : (g + 1) * 128],
                        rhs=w12_sb[:, k, n * 512 : (n + 1) * 512],
                        start=start and (k == 0),
                        stop=stop and (k == KH - 1),
                    )

            def emit_evict_g(tb, n, g, p2s, tiles):
                t0 = tb * TB
                o = outp.tile([128, 512], FP32, name=f"o_{tb}_{n}_{g}", tag="o")
                nc.vector.scalar_tensor_tensor(
                    out=o,
                    in0=p2s[g],
                    scalar=1.0 / S_W,
                    in1=tiles["r32"][:, g, n * 512 : (n + 1) * 512],
                    op0=ALU.mult,
                    op1=ALU.add,
                )
                nc.gpsimd.dma_start(
                    out=out[t0 + g * 128 : t0 + (g + 1) * 128, n * 512 : (n + 1) * 512],
                    in_=o,
                )

            # ---- preamble: LN for block 0 (per-g fast path: each token group
            # flows dma -> stats -> rstd -> x0 -> transposes independently so
            # the PE starts ~18us in), w1 preload split in column halves so
            # GEMM1 m=0 can start as soon as the first half lands ----
            w1_sb = const.tile([128, KH, I], FP8, name="w1_sb")
            w2_sb = const.tile([128, MI, H], FP8, name="w2_sb")
            w12_sb = (
                const.tile([128, KH, H], BF16, name="w12_sb") if use_w12 else None
            )
            st0 = ln_alloc(0)
            for g in range(G):
                ln_chunk(0, g, st0)
                ln_finish(0, st0, gs=[g])
                ln_transposes(0, st0, gs=[g])
            for k in range(KH):
                nc.sync.dma_start(
                    out=w1_sb[:, k, : I // 2], in_=w1[k * 128 : (k + 1) * 128, : I // 2]
                )
            for k in range(KH):
                nc.sync.dma_start(
                    out=w1_sb[:, k, I // 2 :], in_=w1[k * 128 : (k + 1) * 128, I // 2 :]
                )
            for ks in range(4):
                nc.sync.dma_start(
                    out=w2_sb[:, ks * 8 : (ks + 1) * 8, :],
                    in_=w2[ks * 8 * 128 : (ks + 1) * 8 * 128, :].rearrange(
                        "(k p) h -> p k h", p=128
                    ),
                )
            if use_w12:
                nc.sync.dma_start(
                    out=w12_sb,
                    in_=w12[:, :].rearrange("(k p) h -> p k h", p=128),
                )
            tiles = {"xT8": st0["xT8"], "xTb": st0["xTb"], "r32": st0["r32"]}

            # ---- main block loop ----
            for tb in range(n_blocks):
                tiles["hT"] = blk1.tile([128, MI, TB], FP8, name=f"hT_{tb}", tag="hT")
                st_next = ln_alloc(tb + 1) if tb + 1 < n_blocks else None
                # n=0 accumulation group: GEMM1 m-pairs interleaved with the
                # GEMM2 DR k-pairs lagging one m-pair behind (so the PE never
                # waits on the gelu/g* chain), W12 term appended (stop) in
                # g-major order so the psum banks close staggered.
                p2s0 = [
                    ps2.tile([128, 512], FP32, name=f"p2_{tb}_0_{g}", tag="p2")
                    for g in range(G)
                ]
                for mp in range(MI // 2 + 1):
                    if mp < MI // 2:
                        emit_g1_m(tb, 2 * mp, tiles)
                        emit_g1_m(tb, 2 * mp + 1, tiles)
                    if mp >= 1:
                        emit_g2_dr_k(
                            tb, 0, mp - 1, p2s0, tiles,
                            start=(mp == 1),
                            stop=(not use_w12) and (mp == MI // 2),
                        )
                    if st_next is not None and mp % 4 == 3:
                        ln_chunk(tb + 1, mp // 4, st_next)
                if st_next is not None:
                    ln_finish(tb + 1, st_next)
                for g in range(G):
                    if use_w12:
                        emit_w12_g(tb, 0, g, p2s0, tiles, start=False, stop=True)
                    emit_evict_g(tb, 0, g, p2s0, tiles)
                # n=1 group: W12 first (start), then transposes for block tb+1,
                # then the DR k-pairs in g-major order with staggered evicts so
                # the DVE is not head-of-line blocked at the block boundary.
                p2s1 = [
                    ps2.tile([128, 512], FP32, name=f"p2_{tb}_1_{g}", tag="p2")
                    for g in range(G)
                ]
                if use_w12:
                    for g in range(G):
                        emit_w12_g(tb, 1, g, p2s1, tiles, start=True, stop=False)
                if st_next is not None:
                    ln_transposes(tb + 1, st_next)
                for g in range(G):
                    hT = tiles["hT"]
                    for k in range(MI // 2):
                        nc.tensor.matmul(
                            p2s1[g],
                            lhsT=hT[:, 2 * k : 2 * k + 2, g * 128 : (g + 1) * 128],
                            rhs=w2_sb[:, 2 * k : 2 * k + 2, 512:1024],
                            start=(not use_w12) and (k == 0),
                            stop=(k == MI // 2 - 1),
                            perf_mode=DR,
                        )
                    emit_evict_g(tb, 1, g, p2s1, tiles)
                if st_next is not None:
                    tiles = {
                        "xT8": st_next["xT8"],
                        "xTb": st_next["xTb"],
                        "r32": st_next["r32"],
                    }

    return nc


def _prep_inputs(input, residual, bias, attn_nw, attn_nb, inter_w, inter_b, output_w, output_b, use_w12=USE_W12):
    """Host-side preprocessing: fold bias into the input stream and the LN
    affine into W1/b1, scale + cast weights to fp8 e4m3 (clip to +-240: TRN
    e4m3 overflows to inf), precompute W12 = W1'@W2 in bf16, shard tokens."""
    f8 = ml_dtypes.float8_e4m3
    bf = ml_dtypes.bfloat16
    biasf = np.asarray(bias, np.float32)
    x2 = np.ascontiguousarray(
        np.asarray(input, np.float32).reshape(NTOK, H) + biasf
    )
    r2 = np.ascontiguousarray(np.asarray(residual, np.float32).reshape(NTOK, H))
    gamma = np.asarray(attn_nw, np.float64)
    beta = np.asarray(attn_nb, np.float64)
    w1f = np.asarray(inter_w, np.float64)
    w2f = np.asarray(output_w, np.float64)
    w1p = gamma[:, None] * w1f
    w1b = np.ascontiguousarray(
        np.clip(w1p * S_W, -240, 240).astype(np.float32).astype(f8)
    )
    b1p = (np.asarray(inter_b, np.float64) + beta @ w1f).astype(np.float32)
    b1c = np.ascontiguousarray(b1p.reshape(MI, 128).T)
    w2b = np.ascontiguousarray(
        np.clip(w2f * S_W, -240, 240).astype(np.float32).astype(f8)
    )
    b2f = np.asarray(output_b, np.float32)

    in_maps = []
    for c in range(N_CORES):
        sl = slice(c * T, (c + 1) * T)
        im = {
            "xin": x2[sl],
            "res": r2[sl],
            "w1": w1b,
            "w2": w2b,
            "b1c": b1c,
            "b2v": b2f,
        }
        in_maps.append(im)
    if use_w12:
        w12b = np.ascontiguousarray(
            ((w1p @ w2f) * (S_W / (2 * S_X))).astype(np.float32).astype(bf)
        )
        for im in in_maps:
            im["w12"] = w12b
    return in_maps


def _run(inputs, trace=False, **kwargs):
    in_maps = _prep_inputs(
        inputs["input"],
        inputs["residual"],
        inputs["bias"],
        inputs["attn_nw"],
        inputs["attn_nb"],
        inputs["inter_w"],
        inputs["inter_b"],
        inputs["output_w"],
        inputs["output_b"],
    )
    nc = _build()
    _split_multiwait_instructions(nc)
    r = run_bass_kernel_spmd(nc, in_maps, list(range(N_CORES)), trace=trace, **kwargs)
    outs = [r.results[c]["out"] for c in range(N_CORES)]
    full = np.concatenate(outs, axis=0).reshape(B, S, H).astype(np.float32)
    return full, r


def kernel(**inputs):
    out, _ = _run(inputs, trace=False)
    return out


if __name__ == "__main__":
    nc = _build(1)
    print("built 1-block variant ok:", len(nc.m.functions[0].blocks))
